# revision 1
# baseline (speedup 1.0000x reference)
"""DTW loss kernel for Trainium2 (8 NeuronCores, Bass/Tile).

Strategy
--------
reference: C[b,i,j] = ||s1[b,i]-s2[b,j]||^2 ; DTW DP over [512,512]; return
mean_b sqrt(DTW[b,-1,-1]).

Meet-in-the-middle: any monotone DTW path crosses the row-255/256 boundary
exactly once, so DTW_end = min_j F[255,j] + min(B[256,j], B[256,j+1]) where F
is the forward DP over rows 0..255 and B the backward DP (a forward DP on the
reversed sequences). Each core handles 16 batch elements * 2 directions = 32
independent half-DPs ("virtual batches", vb) of 256 rows.

DP rows are computed with tensor_tensor_scan (state = min(m[j], state) + c[j])
plus one scalar_tensor_tensor for m[j] = min(prev[j], prev[j-1]). To shorten
the serial free-dim, a 2-block wavefront runs on 64 partitions = (q, vb),
q in {0,1}: at superstep s lane (0,vb) scans row s cols [0,256) and lane
(1,vb) scans row s-1 cols [256,512). Block carries ride in column 0 of the
row tile: one [32,1] copy per superstep moves lane-q0's tail both into the
scan's per-partition `initial` AP and into the m-prep's j-1 edge slot.

The cost rows are made on the PE in bf16: C[vb,i,j] = u[vb,i,:]@v[vb,j,:]
with u = [-2*s1, 1, |s1|^2], v = [s2, |s2|^2, 1] (K=18), batched over vb via
block-diagonal weights (8 chunks of 4 vb, each vb padded to a 32-partition
K-slot so compute-engine partition offsets stay 32-aligned). GPSIMD casts the
compact f32 u into the bf16 weight tiles; the scalar engine gathers psum
[32,256] pieces into the wavefront layout.
"""

import numpy as np

B = 128
L1 = 512
L2 = 512
D = 16
N_CORES = 8
PER_CORE = B // N_CORES  # 16
VB = 2 * PER_CORE  # 32 virtual batches (fwd+bwd)
R = L1 // 2  # 256 rows per half-DP
KAUG = D + 2  # 18
NCHUNK = 5  # matmul chunks of up to 7 vb, K rows = 7*18 = 126 (unpadded)
KCH = 126  # K rows per chunk
IBLK = 4  # DP rows per psum block
NBLK = R // IBLK  # 64
EIGHTH = 8  # psum blocks per weight-staging buffer
NQ = 3  # wavefront j-blocks
W = 172  # block width (3*172 = 516; q2 has 4 virtual pad cols)
W2 = L2 - 2 * W  # 168 real cols in q2's block
NSS = R + 4  # 260 supersteps (q1 lags 2, q2 lags 4)
BIG = 1e30

_CACHE = {}


def _emit(tc, v_c, w_c, out_rows):
    import concourse.bass as bass  # noqa: F401
    from concourse import mybir

    F32 = mybir.dt.float32
    Alu = mybir.AluOpType
    nc = tc.nc

    with (
        tc.tile_pool(name="singles", bufs=1) as singles,
        tc.tile_pool(name="wpool", bufs=12) as wpool,
        tc.tile_pool(name="psum", bufs=4, space="PSUM") as psum_pool,
    ):
        BF16 = mybir.dt.bfloat16
        # --- persistent tiles ---
        rhs = [singles.tile([KCH, L2], BF16, tag=f"rhs{g}", name=f"rhs{g}") for g in range(NCHUNK)]
        bigm = singles.tile([NQ * VB, W], F32, tag="bigm", name="bigm")
        init0 = singles.tile([NQ * VB, 1], F32, tag="init0", name="init0")
        new = [singles.tile([NQ * VB, W + 1], F32, tag=f"new{p}", name=f"new{p}") for p in range(4)]
        mm = [singles.tile([NQ * VB, W], F32, tag=f"m{p}", name=f"m{p}") for p in range(2)]
        cc = [singles.tile([NQ * VB, W], F32, tag=f"c{p}", name=f"c{p}") for p in range(4)]

        # --- prologue ---
        nc.vector.memset(bigm, BIG)
        nc.vector.memset(init0, 0.0)
        for p in range(4):
            nc.vector.memset(new[p][:, 0:1], BIG)
        for p in range(4):
            nc.vector.memset(cc[p], 0.0)
        for g in range(NCHUNK):
            nc.sync.dma_start(out=rhs[g], in_=v_c[g])

        psum_tiles = {}

        def emit_block(t):
            pt = psum_pool.tile([128, L2], F32, tag="pt", name=f"pt{t}")
            for g in range(NCHUNK):
                w = wpool.tile([KCH, 128], BF16, tag="w", name=f"w{t}_{g}")
                nc.sync.dma_start(out=w, in_=w_c[t, g])
                nc.tensor.matmul(
                    out=pt,
                    lhsT=w,
                    rhs=rhs[g],
                    start=(g == 0),
                    stop=(g == NCHUNK - 1),
                )
            psum_tiles[t] = pt

        # --- wavefront: superstep s: lane q -> row s-2q cols [W*q, W*q+W)
        #     (q2's last 4 cols are virtual pads: c=0, outputs unused) ---
        for s in range(NSS):
            if s % IBLK == 0 and s // IBLK < NBLK:
                emit_block(s // IBLK)
            c_s = cc[s % 4]
            if s < R:
                pt = psum_tiles[s // IBLK]
                nc.scalar.copy(
                    out=c_s[0:VB, :],
                    in_=pt[32 * (s % IBLK) : 32 * (s % IBLK) + 32, 0:W],
                )
            if 2 <= s < R + 2:
                ptm = psum_tiles[(s - 2) // IBLK]
                nc.scalar.copy(
                    out=c_s[VB : 2 * VB, :],
                    in_=ptm[32 * ((s - 2) % IBLK) : 32 * ((s - 2) % IBLK) + 32, W : 2 * W],
                )
            if 4 <= s < R + 4:
                pt2 = psum_tiles[(s - 4) // IBLK]
                nc.scalar.copy(
                    out=c_s[2 * VB : 2 * VB + VB, 0:W2],
                    in_=pt2[32 * ((s - 4) % IBLK) : 32 * ((s - 4) % IBLK) + 32, 2 * W : L2],
                )
            nb = new[s % 4]
            if s == 0:
                d0 = bigm
                ini = init0[:, 0:1]
            else:
                pb = new[(s - 1) % 4]
                if s >= 2:
                    nc.gpsimd.tensor_copy(
                        out=nb[VB : 2 * VB, 0:1],
                        in_=new[(s - 2) % 4][0:VB, W : W + 1],
                    )
                if s >= 4:
                    nc.gpsimd.tensor_copy(
                        out=nb[2 * VB : 3 * VB, 0:1],
                        in_=new[(s - 2) % 4][VB : 2 * VB, W : W + 1],
                    )
                mb = mm[s % 2]
                nc.vector.scalar_tensor_tensor(
                    out=mb, in0=pb[:, 1 : W + 1], scalar=0.0,
                    in1=pb[:, 0:W], op0=Alu.bypass, op1=Alu.min,
                )
                if s == 2:
                    nc.vector.memset(mb[VB : 2 * VB, :], BIG)
                if s == 4:
                    nc.vector.memset(mb[2 * VB : 3 * VB, :], BIG)
                d0 = mb
                ini = nb[:, 0:1]
            nc.vector.tensor_tensor_scan(
                out=nb[:, 1 : W + 1], data0=d0, data1=c_s, initial=ini,
                op0=Alu.min, op1=Alu.add,
            )
        nc.sync.dma_start(
            out=out_rows[:, 0:W], in_=new[(R - 1) % 4][0:VB, 1 : W + 1]
        )
        nc.sync.dma_start(
            out=out_rows[:, W : 2 * W], in_=new[(R + 1) % 4][VB : 2 * VB, 1 : W + 1]
        )
        nc.sync.dma_start(
            out=out_rows[:, 2 * W : L2],
            in_=new[(R + 3) % 4][2 * VB : 3 * VB, 1 : W2 + 1],
        )


def _build():
    import concourse.bacc as bacc
    import concourse.tile as tile
    from concourse import mybir

    F32 = mybir.dt.float32
    BF16 = mybir.dt.bfloat16
    nc = bacc.Bacc()
    v_c = nc.dram_tensor("v_c", [NCHUNK, KCH, L2], BF16, kind="ExternalInput")[:]
    w_c = nc.dram_tensor("w_c", [NBLK, NCHUNK, KCH, 128], BF16, kind="ExternalInput")[:]
    out_rows = nc.dram_tensor("out_rows", [VB, L2], F32, kind="ExternalOutput")[:]
    with tile.TileContext(nc) as tc:
        _emit(tc, v_c, w_c, out_rows)
    nc.compile()
    return nc


def _host_prep(s1, s2):
    """Build per-core v_c [5,126,512] (bf16 rhs chunks) and the full
    block-diagonal weight tensor w_c [64,5,126,128] (bf16)."""
    import ml_dtypes

    BF = ml_dtypes.bfloat16
    s1 = np.ascontiguousarray(s1, dtype=np.float32)
    s2 = np.ascontiguousarray(s2, dtype=np.float32)
    in_maps = []
    for c in range(N_CORES):
        s1c = s1[c * PER_CORE : (c + 1) * PER_CORE]  # [16, 512, 16]
        s2c = s2[c * PER_CORE : (c + 1) * PER_CORE]
        s1v = np.concatenate([s1c[:, :R], s1c[:, ::-1][:, :R]], axis=0)  # [32,256,16]
        s2v = np.concatenate([s2c, s2c[:, ::-1]], axis=0)  # [32,512,16]
        u = np.empty((VB, R, KAUG), np.float32)
        u[:, :, :D] = -2.0 * s1v
        u[:, :, D] = 1.0
        u[:, :, D + 1] = (s1v * s1v).sum(-1)
        v = np.empty((VB, L2, KAUG), np.float32)
        v[:, :, :D] = s2v
        v[:, :, D] = (s2v * s2v).sum(-1)
        v[:, :, D + 1] = 1.0
        u = u.astype(BF)
        vch = np.zeros((NCHUNK, KCH, L2), BF)
        wch = np.zeros((NBLK, NCHUNK, KCH, 128), BF)
        for g in range(NCHUNK):
            for vl in range(min(7, VB - 7 * g)):
                vb = 7 * g + vl
                vch[g, vl * KAUG : (vl + 1) * KAUG, :] = v[vb].T
                # w[t, g, vl*18+d, il*32+vb] = u[vb, 4t+il, d]
                wch[:, g, vl * KAUG : (vl + 1) * KAUG, vb::VB] = (
                    u[vb].reshape(NBLK, IBLK, KAUG).transpose(0, 2, 1)
                )
        in_maps.append(
            {
                "v_c": vch,
                "w_c": wch,
            }
        )
    return in_maps


def _combine(outs):
    """outs: list of [VB, 512] final-row arrays per core -> scalar loss."""
    vals = np.empty(B, np.float64)
    for c in range(N_CORES):
        rows = outs[c]
        for bl in range(PER_CORE):
            F = rows[bl].astype(np.float64)
            Brow = rows[PER_CORE + bl][::-1].astype(np.float64)
            Bnext = np.concatenate([Brow[1:], [np.inf]])
            vals[c * PER_CORE + bl] = np.min(F + np.minimum(Brow, Bnext))
    return np.float32(np.mean(np.sqrt(vals)))


def kernel(s1_batch, s2_batch):
    from concourse import bass_utils

    if "nc" not in _CACHE:
        _CACHE["nc"] = _build()
    nc = _CACHE["nc"]
    in_maps = _host_prep(np.asarray(s1_batch), np.asarray(s2_batch))
    kw = {}
    if _CACHE.get("trace"):
        kw = dict(trace=True, trace_cores=_CACHE.get("trace_cores", [0]),
                  tmpdir=_CACHE.get("tmpdir"))
    res = bass_utils.run_bass_kernel_spmd(
        nc, in_maps, core_ids=list(range(N_CORES)), **kw
    )
    if res.exec_time_ns is not None:
        _CACHE["exec_time_ns"] = res.exec_time_ns
    _CACHE["last_results"] = res
    outs = [r["out_rows"] for r in res.results]
    return _combine(outs)



# revision 2
# speedup vs baseline: 1.0099x; 1.0099x over previous
"""DTW loss kernel for Trainium2 (8 NeuronCores, Bass/Tile).

Strategy
--------
reference: C[b,i,j] = ||s1[b,i]-s2[b,j]||^2 ; DTW DP over [512,512]; return
mean_b sqrt(DTW[b,-1,-1]).

Meet-in-the-middle: any monotone DTW path crosses the row-255/256 boundary
exactly once, so DTW_end = min_j F[255,j] + min(B[256,j], B[256,j+1]) where F
is the forward DP over rows 0..255 and B the backward DP (a forward DP on the
reversed sequences). Each core handles 16 batch elements * 2 directions = 32
independent half-DPs ("virtual batches", vb) of 256 rows.

DP rows are computed with tensor_tensor_scan (state = min(m[j], state) + c[j])
plus one scalar_tensor_tensor for m[j] = min(prev[j], prev[j-1]). To shorten
the serial free-dim, a 2-block wavefront runs on 64 partitions = (q, vb),
q in {0,1}: at superstep s lane (0,vb) scans row s cols [0,256) and lane
(1,vb) scans row s-1 cols [256,512). Block carries ride in column 0 of the
row tile: one [32,1] copy per superstep moves lane-q0's tail both into the
scan's per-partition `initial` AP and into the m-prep's j-1 edge slot.

DP row storage is fp16 (the scan's internal state stays fp32; outputs are
downcast per element). Scan operand APs are kept 4-byte aligned (row tiles
have a 2-col header) so the DVE 2x_1p perf mode can kick in for 16-bit data.

The cost rows are made on the PE in bf16: C[vb,i,j] = u[vb,i,:]@v[vb,j,:]
with u = [-2*s1, 1, |s1|^2], v = [s2, |s2|^2, 1] (K=18), batched over vb via
block-diagonal weights (8 chunks of 4 vb, each vb padded to a 32-partition
K-slot so compute-engine partition offsets stay 32-aligned). All 5 K-chunk
weight tiles for a psum block load with ONE dma (dram layout [KCH, 5*128])
to keep the sync sequencer's descriptor-generation off the critical path.
The scalar engine gathers psum [32,256] pieces into the wavefront layout,
casting fp32 -> fp16.
"""

import numpy as np

B = 128
L1 = 512
L2 = 512
D = 16
N_CORES = 8
PER_CORE = B // N_CORES  # 16
VB = 2 * PER_CORE  # 32 virtual batches (fwd+bwd)
R = L1 // 2  # 256 rows per half-DP
KAUG = D + 2  # 18
NCHUNK = 5  # matmul chunks of up to 7 vb, K rows = 7*18 = 126 (unpadded)
KCH = 126  # K rows per chunk
IBLK = 4  # DP rows per psum block
NBLK = R // IBLK  # 64
NQ = 3  # wavefront j-blocks
W = 172  # block width (3*172 = 516; q2 has 4 virtual pad cols)
W2 = L2 - 2 * W  # 168 real cols in q2's block
NSS = R + 4  # 260 supersteps (q1 lags 2, q2 lags 4)
BIG = 60000.0  # fp16-safe sentinel (max fp16 = 65504)

_CACHE = {}


def _emit(tc, v_c, w_c, out_rows):
    import concourse.bass as bass  # noqa: F401
    from concourse import mybir

    F32 = mybir.dt.float32
    F16 = mybir.dt.float16
    Alu = mybir.AluOpType
    nc = tc.nc

    with (
        tc.tile_pool(name="singles", bufs=1) as singles,
        tc.tile_pool(name="wpool", bufs=4) as wpool,
        tc.tile_pool(name="psum", bufs=4, space="PSUM") as psum_pool,
    ):
        BF16 = mybir.dt.bfloat16
        # --- persistent tiles ---
        rhs = [singles.tile([KCH, L2], BF16, tag=f"rhs{g}", name=f"rhs{g}") for g in range(NCHUNK)]
        bigm = singles.tile([NQ * VB, W], F16, tag="bigm", name="bigm")
        init0 = singles.tile([NQ * VB, 1], F16, tag="init0", name="init0")
        # row tiles: col 0 = carry slot, col 1 = m-prep edge, cols 2:W+2 = row
        new = [singles.tile([NQ * VB, W + 2], F16, tag=f"new{p}", name=f"new{p}") for p in range(4)]
        mm = [singles.tile([NQ * VB, W], F16, tag=f"m{p}", name=f"m{p}") for p in range(2)]
        cc = [singles.tile([NQ * VB, W], F16, tag=f"c{p}", name=f"c{p}") for p in range(4)]

        # --- prologue ---
        nc.vector.memset(bigm, BIG)
        nc.vector.memset(init0, 0.0)
        for p in range(4):
            nc.vector.memset(new[p][:, 0:2], BIG)
        for p in range(4):
            nc.vector.memset(cc[p], 0.0)
        for g in range(NCHUNK):
            nc.sync.dma_start(out=rhs[g], in_=v_c[g])

        psum_tiles = {}

        def emit_block(t):
            pt = psum_pool.tile([128, L2], F32, tag="pt", name=f"pt{t}")
            w = wpool.tile([KCH, NCHUNK * 128], BF16, tag="w", name=f"w{t}")
            nc.sync.dma_start(out=w, in_=w_c[t])
            for g in range(NCHUNK):
                nc.tensor.matmul(
                    out=pt,
                    lhsT=w[:, g * 128 : (g + 1) * 128],
                    rhs=rhs[g],
                    start=(g == 0),
                    stop=(g == NCHUNK - 1),
                )
            psum_tiles[t] = pt

        # --- wavefront: superstep s: lane q -> row s-2q cols [W*q, W*q+W)
        #     (q2's last 4 cols are virtual pads: c=0, outputs unused) ---
        for s in range(NSS):
            if s % IBLK == 0 and s // IBLK < NBLK:
                emit_block(s // IBLK)
            c_s = cc[s % 4]
            if s < R:
                pt = psum_tiles[s // IBLK]
                nc.scalar.copy(
                    out=c_s[0:VB, :],
                    in_=pt[32 * (s % IBLK) : 32 * (s % IBLK) + 32, 0:W],
                )
            if 2 <= s < R + 2:
                ptm = psum_tiles[(s - 2) // IBLK]
                nc.scalar.copy(
                    out=c_s[VB : 2 * VB, :],
                    in_=ptm[32 * ((s - 2) % IBLK) : 32 * ((s - 2) % IBLK) + 32, W : 2 * W],
                )
            if 4 <= s < R + 4:
                pt2 = psum_tiles[(s - 4) // IBLK]
                nc.scalar.copy(
                    out=c_s[2 * VB : 2 * VB + VB, 0:W2],
                    in_=pt2[32 * ((s - 4) % IBLK) : 32 * ((s - 4) % IBLK) + 32, 2 * W : L2],
                )
            nb = new[s % 4]
            if s == 0:
                d0 = bigm
                ini = init0[:, 0:1]
            else:
                pb = new[(s - 1) % 4]
                if s >= 2:
                    nc.gpsimd.tensor_copy(
                        out=nb[VB : 2 * VB, 0:2],
                        in_=new[(s - 2) % 4][0:VB, W : W + 2],
                    )
                if s >= 4:
                    nc.gpsimd.tensor_copy(
                        out=nb[2 * VB : 3 * VB, 0:2],
                        in_=new[(s - 2) % 4][VB : 2 * VB, W : W + 2],
                    )
                mb = mm[s % 2]
                nc.vector.scalar_tensor_tensor(
                    out=mb, in0=pb[:, 2 : W + 2], scalar=0.0,
                    in1=pb[:, 1 : W + 1], op0=Alu.bypass, op1=Alu.min,
                )
                if s == 2:
                    nc.vector.memset(mb[VB : 2 * VB, :], BIG)
                if s == 4:
                    nc.vector.memset(mb[2 * VB : 3 * VB, :], BIG)
                d0 = mb
                ini = nb[:, 1:2]
            nc.vector.tensor_tensor_scan(
                out=nb[:, 2 : W + 2], data0=d0, data1=c_s, initial=ini,
                op0=Alu.min, op1=Alu.add,
            )
        nc.sync.dma_start(
            out=out_rows[:, 0:W], in_=new[(R - 1) % 4][0:VB, 2 : W + 2]
        )
        nc.sync.dma_start(
            out=out_rows[:, W : 2 * W], in_=new[(R + 1) % 4][VB : 2 * VB, 2 : W + 2]
        )
        nc.sync.dma_start(
            out=out_rows[:, 2 * W : L2],
            in_=new[(R + 3) % 4][2 * VB : 3 * VB, 2 : W2 + 2],
        )


def _build():
    import concourse.bacc as bacc
    import concourse.tile as tile
    from concourse import mybir

    F16 = mybir.dt.float16
    BF16 = mybir.dt.bfloat16
    nc = bacc.Bacc()
    v_c = nc.dram_tensor("v_c", [NCHUNK, KCH, L2], BF16, kind="ExternalInput")[:]
    w_c = nc.dram_tensor("w_c", [NBLK, KCH, NCHUNK * 128], BF16, kind="ExternalInput")[:]
    out_rows = nc.dram_tensor("out_rows", [VB, L2], F16, kind="ExternalOutput")[:]
    with tile.TileContext(nc) as tc:
        _emit(tc, v_c, w_c, out_rows)
    nc.compile()
    return nc


def _host_prep(s1, s2):
    """Build per-core v_c [5,126,512] (bf16 rhs chunks) and the full
    block-diagonal weight tensor w_c [64,126,640] (bf16, 5 K-chunks of a
    psum block side by side so one DMA loads them all)."""
    import ml_dtypes

    BF = ml_dtypes.bfloat16
    s1 = np.ascontiguousarray(s1, dtype=np.float32)
    s2 = np.ascontiguousarray(s2, dtype=np.float32)
    in_maps = []
    for c in range(N_CORES):
        s1c = s1[c * PER_CORE : (c + 1) * PER_CORE]  # [16, 512, 16]
        s2c = s2[c * PER_CORE : (c + 1) * PER_CORE]
        s1v = np.concatenate([s1c[:, :R], s1c[:, ::-1][:, :R]], axis=0)  # [32,256,16]
        s2v = np.concatenate([s2c, s2c[:, ::-1]], axis=0)  # [32,512,16]
        u = np.empty((VB, R, KAUG), np.float32)
        u[:, :, :D] = -2.0 * s1v
        u[:, :, D] = 1.0
        u[:, :, D + 1] = (s1v * s1v).sum(-1)
        v = np.empty((VB, L2, KAUG), np.float32)
        v[:, :, :D] = s2v
        v[:, :, D] = (s2v * s2v).sum(-1)
        v[:, :, D + 1] = 1.0
        u = u.astype(BF)
        vch = np.zeros((NCHUNK, KCH, L2), BF)
        wch = np.zeros((NBLK, NCHUNK, KCH, 128), BF)
        for g in range(NCHUNK):
            for vl in range(min(7, VB - 7 * g)):
                vb = 7 * g + vl
                vch[g, vl * KAUG : (vl + 1) * KAUG, :] = v[vb].T
                # w[t, g, vl*18+d, il*32+vb] = u[vb, 4t+il, d]
                wch[:, g, vl * KAUG : (vl + 1) * KAUG, vb::VB] = (
                    u[vb].reshape(NBLK, IBLK, KAUG).transpose(0, 2, 1)
                )
        w2 = np.ascontiguousarray(wch.transpose(0, 2, 1, 3)).reshape(
            NBLK, KCH, NCHUNK * 128
        )
        in_maps.append(
            {
                "v_c": vch,
                "w_c": w2,
            }
        )
    return in_maps


def _combine(outs):
    """outs: list of [VB, 512] final-row arrays per core -> scalar loss."""
    vals = np.empty(B, np.float64)
    for c in range(N_CORES):
        rows = outs[c]
        for bl in range(PER_CORE):
            F = rows[bl].astype(np.float64)
            Brow = rows[PER_CORE + bl][::-1].astype(np.float64)
            Bnext = np.concatenate([Brow[1:], [np.inf]])
            vals[c * PER_CORE + bl] = np.min(F + np.minimum(Brow, Bnext))
    return np.float32(np.mean(np.sqrt(vals)))


def kernel(s1_batch, s2_batch):
    from concourse import bass_utils

    if "nc" not in _CACHE:
        _CACHE["nc"] = _build()
    nc = _CACHE["nc"]
    in_maps = _host_prep(np.asarray(s1_batch), np.asarray(s2_batch))
    kw = {}
    if _CACHE.get("trace"):
        kw = dict(trace=True, trace_cores=_CACHE.get("trace_cores", [0]),
                  tmpdir=_CACHE.get("tmpdir"))
    res = bass_utils.run_bass_kernel_spmd(
        nc, in_maps, core_ids=list(range(N_CORES)), **kw
    )
    if res.exec_time_ns is not None:
        _CACHE["exec_time_ns"] = res.exec_time_ns
    _CACHE["last_results"] = res
    outs = [np.asarray(r["out_rows"], dtype=np.float32) for r in res.results]
    return _combine(outs)


# revision 10
# speedup vs baseline: 1.2593x; 1.2469x over previous
"""DTW loss kernel for Trainium2 (8 NeuronCores, Bass/Tile).

Strategy
--------
reference: C[b,i,j] = ||s1[b,i]-s2[b,j]||^2 ; DTW DP over [512,512]; return
mean_b sqrt(DTW[b,-1,-1]).

Meet-in-the-middle: any monotone DTW path crosses the row-255/256 boundary
exactly once, so DTW_end = min_j F[255,j] + min(B[256,j], B[256,j+1]) where F
is the forward DP over rows 0..255 and B the backward DP (a forward DP on the
reversed sequences). Each core handles 16 batch elements * 2 directions = 32
independent half-DPs ("virtual batches", vb) of 256 rows. A 3-block column
wavefront runs on 96 partitions = (q, vb): at superstep s lane (q,vb) scans
row s-2q over cols [W*q, W*q+W).

Each DP row is ONE custom DVE instruction (full-rate 1 elem/cycle, vs the
stock tensor_tensor_scan's half-rate bubble path) via a change of variables:
with S_j = prefix-sum(c), the recurrence state_j = min(prev_j, prev_{j-1},
state_{j-1}) + c_j becomes a pure running min state'_j = min(state'_{j-1},
m_j - S_{j-1}) with state_j = state'_j + S_j. Feeding the op an overlapping
window AP (prev_{j-1}, prev_j interleaved, stride-2 pairs) and a
zero-interleaved cost stream makes m_j's two terms arrive as separate stream
elements, so the body is uniform:
    S = scan(ADD, Src1); b = Src0 - S + Src1
    r = scan(MIN, b, init=C0); out = r + S
Row tiles store values at odd columns (col 1 = left-edge/carry slot), so one
op's output is directly the next op's window input. The per-lane carry init
C0 rides in a [96,1] column AP written by one gpsimd copy per superstep.

The cost rows are made on the PE in bf16: C[vb,i,j] = u[vb,i,:]@v[vb,j,:]
with u = [-2*s1, 1, |s1|^2], v = [s2, |s2|^2, 1] (K=18), batched over vb via
block-diagonal weights. All 5 K-chunk weight tiles for a psum block load with
ONE dma (dram layout [KCH, 5*128]) to keep the sync sequencer off the
critical path. The scalar engine gathers q0/q1 psum pieces into the
wavefront cost tiles (strided odd-column writes); the vector engine gathers
q2's.
"""

import numpy as np

B = 128
L1 = 512
L2 = 512
D = 16
N_CORES = 8
PER_CORE = B // N_CORES  # 16
VB = 2 * PER_CORE  # 32 virtual batches (fwd+bwd)
R = L1 // 2  # 256 rows per half-DP
KAUG = D + 2  # 18
NCHUNK = 5  # matmul chunks of up to 7 vb, K rows = 7*18 = 126 (unpadded)
KCH = 126  # K rows per chunk
IBLK = 4  # DP rows per psum block
NBLK = R // IBLK  # 64
NQ = 3  # wavefront j-blocks
W = 172  # block width (3*172 = 516; q2 has 4 virtual pad cols)
W2 = L2 - 2 * W  # 168 real cols in q2's block
NSS = R + 4  # 260 supersteps (q1 lags 2, q2 lags 4)
BIG = 1e30
CW = 2 + 2 * W  # row/cost tile columns: [unused, edge, interleaved 2*W]
SEG = 2 * W  # interleaved segment width (344)

_CACHE = {}


def _get_dtw_op():
    """Register (once) and return the fused DTW-row custom DVE op."""
    if "dtw_op" in _CACHE:
        return _CACHE["dtw_op"]
    import concourse.dve_ops as dve_ops_mod
    from concourse import dve_spec
    from concourse.dve_spec import Spec, Src0, Src1, C0, AluOp, scan, lower
    from concourse.dve_ops import DveOp
    from concourse.dve_uop import DveOpSpec

    def mk_scan(op, expr, init=None):
        # Scan.__post_init__ rejects a scan whose expr contains another scan,
        # but the schedule is legal (S at stage 0, b at 1-2, min-combine at 3
        # via CURR_ALU_OUT feedback); construct the node directly.
        s = object.__new__(dve_spec.Scan)
        for k, v in (("op", op), ("expr", expr), ("init", init),
                     ("_subdim_step", None)):
            object.__setattr__(s, k, v)
        return s

    def ref(in0, in1, c0, c1, c2):
        P = in0.shape[0]
        x0 = np.asarray(in0, np.float32).reshape(P, -1)
        x1 = np.asarray(in1, np.float32).reshape(P, -1)
        S = np.cumsum(x1, axis=1, dtype=np.float32)
        b = (x0 - S) + x1
        r = np.minimum.accumulate(np.minimum(b, c0), axis=1)
        return r + S

    S = scan(AluOp.ADD, Src1)
    b = (Src0 - S) + Src1
    r = mk_scan(AluOp.MIN, b, C0)
    spec = Spec(body=r + S, reference=ref)

    name = "DTW_ROW_ANT"
    if name not in dve_ops_mod._SUB_OPCODE_FOR_NAME:
        opcode = max(dve_ops_mod._SUB_OPCODE_FOR_NAME.values()) + 1
        assert opcode < 0x20
        dve_ops_mod._SUB_OPCODE_FOR_NAME[name] = opcode
    opcode = dve_ops_mod._SUB_OPCODE_FOR_NAME[name]
    shas = {}
    for ver in ("v3", "v4"):
        uops = lower(spec, ver=ver)
        shas[ver] = DveOpSpec(
            name=name, opcode=opcode, uops=uops, rd1_en=True
        ).sha(ver)
    op = DveOp(name, spec, subdim=False, uops_sha=shas)
    if all(o.name != name for o in dve_ops_mod.OPS):
        dve_ops_mod.OPS.append(op)
    dve_ops_mod.CUSTOM_DVE_SPECS[name] = spec
    _CACHE["dtw_op"] = op
    return op


def _emit(tc, v_c, w_c, out_rows):
    import concourse.bass as bass  # noqa: F401
    from concourse import mybir

    F32 = mybir.dt.float32
    nc = tc.nc
    dtw_op = _get_dtw_op()

    with (
        tc.tile_pool(name="singles", bufs=1) as singles,
        tc.tile_pool(name="wpool", bufs=6) as wpool,
        tc.tile_pool(name="psum", bufs=6, space="PSUM") as psum_pool,
    ):
        BF16 = mybir.dt.bfloat16
        # --- persistent tiles ---
        rhs = [singles.tile([KCH, L2], BF16, tag=f"rhs{g}", name=f"rhs{g}") for g in range(NCHUNK)]
        bigm = singles.tile([NQ * VB, CW], F32, tag="bigm", name="bigm")
        init0 = singles.tile([NQ * VB, 1], F32, tag="init0", name="init0")
        # row tiles: col 1 = edge/carry slot, odd cols 3,5,..,CW-1 = row values
        new = [singles.tile([NQ * VB, CW], F32, tag=f"new{p}", name=f"new{p}") for p in range(4)]
        NCC = 8  # cost-tile ring; deeper than `new` so gathers can run ahead
        cc = [singles.tile([NQ * VB, CW], F32, tag=f"c{p}", name=f"c{p}") for p in range(NCC)]

        def window(t):
            # [96, W, 2] overlapping pairs (v_{j-1}, v_j) at cols (1+2j, 3+2j)
            ap = t[:, 1:3].copy()
            ap.ap = mybir.VecI64Pair([[CW, NQ * VB], [2, W], [2, 2]])
            return ap

        # --- prologue ---
        nc.vector.memset(bigm, BIG)
        nc.vector.memset(bigm[:, 1:2], 0.0)
        nc.vector.memset(init0, 0.0)
        for p in range(4):
            nc.vector.memset(new[p][:, 1:2], BIG)
        for p in range(NCC):
            nc.vector.memset(cc[p], 0.0)
        for g in range(NCHUNK):
            nc.sync.dma_start(out=rhs[g], in_=v_c[g])

        psum_tiles = {}

        def emit_block(t):
            pt = psum_pool.tile([128, L2], F32, tag="pt", name=f"pt{t}")
            w = wpool.tile([KCH, NCHUNK * 128], BF16, tag="w", name=f"w{t}")
            nc.sync.dma_start(out=w, in_=w_c[t])
            for g in range(NCHUNK):
                nc.tensor.matmul(
                    out=pt,
                    lhsT=w[:, g * 128 : (g + 1) * 128],
                    rhs=rhs[g],
                    start=(g == 0),
                    stop=(g == NCHUNK - 1),
                )
            psum_tiles[t] = pt

        # --- wavefront: superstep s: lane q -> row s-2q cols [W*q, W*q+W) ---
        PREFETCH = 2  # emit matmul blocks this many blocks ahead of consumption
        for t in range(PREFETCH):
            emit_block(t)
        for s in range(NSS):
            if s % IBLK == 0 and s // IBLK + PREFETCH < NBLK:
                emit_block(s // IBLK + PREFETCH)
            c_s = cc[s % NCC]
            if s < R:
                pt = psum_tiles[s // IBLK]
                nc.scalar.copy(
                    out=c_s[0:VB, 3:CW:2],
                    in_=pt[32 * (s % IBLK) : 32 * (s % IBLK) + 32, 0:W],
                )
            if 2 <= s < R + 2:
                ptm = psum_tiles[(s - 2) // IBLK]
                nc.scalar.copy(
                    out=c_s[VB : 2 * VB, 3:CW:2],
                    in_=ptm[32 * ((s - 2) % IBLK) : 32 * ((s - 2) % IBLK) + 32, W : 2 * W],
                )
            if 4 <= s < R + 4:
                pt2 = psum_tiles[(s - 4) // IBLK]
                nc.vector.tensor_copy(
                    out=c_s[2 * VB : 3 * VB, 3 : 3 + 2 * W2 : 2],
                    in_=pt2[32 * ((s - 4) % IBLK) : 32 * ((s - 4) % IBLK) + 32, 2 * W : L2],
                )
            nb = new[s % 4]
            if s == 0:
                pb = bigm
                ini = init0[:, 0:1]
            else:
                pb = new[(s - 1) % 4]
                if s >= 2:
                    nc.gpsimd.tensor_copy(
                        out=nb[VB : 2 * VB, 1:2],
                        in_=new[(s - 2) % 4][0:VB, CW - 1 : CW],
                    )
                if s >= 4:
                    nc.gpsimd.tensor_copy(
                        out=nb[2 * VB : 3 * VB, 1:2],
                        in_=new[(s - 2) % 4][VB : 2 * VB, CW - 1 : CW],
                    )
                if s == 2:
                    nc.vector.memset(pb[VB : 2 * VB, 2:CW], BIG)
                if s == 4:
                    nc.vector.memset(pb[2 * VB : 3 * VB, 2:CW], BIG)
                ini = nb[:, 1:2]
            nc.vector._custom_dve(
                dtw_op,
                out=nb[:, 2:CW],
                in0=window(pb),
                in1=c_s[:, 2:CW],
                s0=ini,
            )
        nc.sync.dma_start(
            out=out_rows[:, 0:SEG], in_=new[(R - 1) % 4][0:VB, 2:CW]
        )
        nc.sync.dma_start(
            out=out_rows[:, SEG : 2 * SEG],
            in_=new[(R + 1) % 4][VB : 2 * VB, 2:CW],
        )
        nc.sync.dma_start(
            out=out_rows[:, 2 * SEG : 3 * SEG],
            in_=new[(R + 3) % 4][2 * VB : 3 * VB, 2:CW],
        )


def _build():
    import concourse.bacc as bacc
    import concourse.tile as tile
    from concourse import mybir

    F32 = mybir.dt.float32
    BF16 = mybir.dt.bfloat16
    nc = bacc.Bacc()
    v_c = nc.dram_tensor("v_c", [NCHUNK, KCH, L2], BF16, kind="ExternalInput")[:]
    w_c = nc.dram_tensor("w_c", [NBLK, KCH, NCHUNK * 128], BF16, kind="ExternalInput")[:]
    out_rows = nc.dram_tensor("out_rows", [VB, 3 * SEG], F32, kind="ExternalOutput")[:]
    with tile.TileContext(nc) as tc:
        _emit(tc, v_c, w_c, out_rows)
    nc.compile()
    return nc


def _host_prep(s1, s2):
    """Build per-core v_c [5,126,512] (bf16 rhs chunks) and the full
    block-diagonal weight tensor w_c [64,126,640] (bf16, 5 K-chunks of a
    psum block side by side so one DMA loads them all)."""
    import ml_dtypes

    BF = ml_dtypes.bfloat16
    s1 = np.ascontiguousarray(s1, dtype=np.float32)
    s2 = np.ascontiguousarray(s2, dtype=np.float32)
    in_maps = []
    for c in range(N_CORES):
        s1c = s1[c * PER_CORE : (c + 1) * PER_CORE]  # [16, 512, 16]
        s2c = s2[c * PER_CORE : (c + 1) * PER_CORE]
        s1v = np.concatenate([s1c[:, :R], s1c[:, ::-1][:, :R]], axis=0)  # [32,256,16]
        s2v = np.concatenate([s2c, s2c[:, ::-1]], axis=0)  # [32,512,16]
        u = np.empty((VB, R, KAUG), np.float32)
        u[:, :, :D] = -2.0 * s1v
        u[:, :, D] = 1.0
        u[:, :, D + 1] = (s1v * s1v).sum(-1)
        v = np.empty((VB, L2, KAUG), np.float32)
        v[:, :, :D] = s2v
        v[:, :, D] = (s2v * s2v).sum(-1)
        v[:, :, D + 1] = 1.0
        u = u.astype(BF)
        vch = np.zeros((NCHUNK, KCH, L2), BF)
        wch = np.zeros((NBLK, NCHUNK, KCH, 128), BF)
        for g in range(NCHUNK):
            for vl in range(min(7, VB - 7 * g)):
                vb = 7 * g + vl
                vch[g, vl * KAUG : (vl + 1) * KAUG, :] = v[vb].T
                # w[t, g, vl*18+d, il*32+vb] = u[vb, 4t+il, d]
                wch[:, g, vl * KAUG : (vl + 1) * KAUG, vb::VB] = (
                    u[vb].reshape(NBLK, IBLK, KAUG).transpose(0, 2, 1)
                )
        w2 = np.ascontiguousarray(wch.transpose(0, 2, 1, 3)).reshape(
            NBLK, KCH, NCHUNK * 128
        )
        in_maps.append(
            {
                "v_c": vch,
                "w_c": w2,
            }
        )
    return in_maps


def _combine(outs):
    """outs: list of [VB, 3*SEG] interleaved final-row arrays per core ->
    scalar loss. Row value j of segment q sits at col q*SEG + 1 + 2j."""
    vals = np.empty(B, np.float64)
    for c in range(N_CORES):
        rows = np.asarray(outs[c], np.float64)
        full = np.empty((VB, L2), np.float64)
        full[:, 0:W] = rows[:, 1 : 2 * W : 2]
        full[:, W : 2 * W] = rows[:, SEG + 1 : SEG + 2 * W : 2]
        full[:, 2 * W : L2] = rows[:, 2 * SEG + 1 : 2 * SEG + 2 * W2 : 2]
        for bl in range(PER_CORE):
            F = full[bl]
            Brow = full[PER_CORE + bl][::-1]
            Bnext = np.concatenate([Brow[1:], [np.inf]])
            vals[c * PER_CORE + bl] = np.min(F + np.minimum(Brow, Bnext))
    return np.float32(np.mean(np.sqrt(vals)))


def kernel(s1_batch, s2_batch):
    from concourse import bass_utils

    if "nc" not in _CACHE:
        _CACHE["nc"] = _build()
    nc = _CACHE["nc"]
    in_maps = _host_prep(np.asarray(s1_batch), np.asarray(s2_batch))
    kw = {}
    if _CACHE.get("trace"):
        kw = dict(trace=True, trace_cores=_CACHE.get("trace_cores", [0]),
                  tmpdir=_CACHE.get("tmpdir"))
    res = bass_utils.run_bass_kernel_spmd(
        nc, in_maps, core_ids=list(range(N_CORES)), **kw
    )
    if res.exec_time_ns is not None:
        _CACHE["exec_time_ns"] = res.exec_time_ns
    _CACHE["last_results"] = res
    outs = [r["out_rows"] for r in res.results]
    return _combine(outs)


# revision 11
# speedup vs baseline: 1.2659x; 1.0053x over previous
"""DTW loss kernel for Trainium2 (8 NeuronCores, Bass/Tile).

Strategy
--------
reference: C[b,i,j] = ||s1[b,i]-s2[b,j]||^2 ; DTW DP over [512,512]; return
mean_b sqrt(DTW[b,-1,-1]).

Meet-in-the-middle: any monotone DTW path crosses the row-255/256 boundary
exactly once, so DTW_end = min_j F[255,j] + min(B[256,j], B[256,j+1]) where F
is the forward DP over rows 0..255 and B the backward DP (a forward DP on the
reversed sequences). Each core handles 16 batch elements * 2 directions = 32
independent half-DPs ("virtual batches", vb) of 256 rows. A 3-block column
wavefront runs on 96 partitions = (q, vb): at superstep s lane (q,vb) scans
row s-2q over cols [W*q, W*q+W).

Each DP row is ONE custom DVE instruction (full-rate 1 elem/cycle, vs the
stock tensor_tensor_scan's half-rate bubble path) via a change of variables:
with S_j = prefix-sum(c), the recurrence state_j = min(prev_j, prev_{j-1},
state_{j-1}) + c_j becomes a pure running min state'_j = min(state'_{j-1},
m_j - S_{j-1}) with state_j = state'_j + S_j. Feeding the op an overlapping
window AP (prev_{j-1}, prev_j interleaved, stride-2 pairs) and a
zero-interleaved cost stream makes m_j's two terms arrive as separate stream
elements, so the body is uniform:
    S = scan(ADD, Src1); b = Src0 - S + Src1
    r = scan(MIN, b, init=C0); out = r + S
Row tiles store values at odd columns (col 1 = left-edge/carry slot), so one
op's output is directly the next op's window input. The per-lane carry init
C0 rides in a [96,1] column AP written by one gpsimd copy per superstep.

The cost rows are made on the PE in bf16: C[vb,i,j] = u[vb,i,:]@v[vb,j,:]
with u = [-2*s1, 1, |s1|^2], v = [s2, |s2|^2, 1] (K=18), batched over vb via
block-diagonal weights. All 5 K-chunk weight tiles for a psum block load with
ONE dma (dram layout [KCH, 5*128]) to keep the sync sequencer off the
critical path. The scalar engine gathers q0/q1 psum pieces into the
wavefront cost tiles (strided odd-column writes); the vector engine gathers
q2's.
"""

import numpy as np

B = 128
L1 = 512
L2 = 512
D = 16
N_CORES = 8
PER_CORE = B // N_CORES  # 16
VB = 2 * PER_CORE  # 32 virtual batches (fwd+bwd)
R = L1 // 2  # 256 rows per half-DP
KAUG = D + 2  # 18
NCHUNK = 5  # matmul chunks of up to 7 vb, K rows = 7*18 = 126 (unpadded)
KCH = 126  # K rows per chunk
IBLK = 4  # DP rows per psum block
NBLK = R // IBLK  # 64
NQ = 3  # wavefront j-blocks
W = 172  # block width (3*172 = 516; q2 has 4 virtual pad cols)
W2 = L2 - 2 * W  # 168 real cols in q2's block
NSS = R + 4  # 260 supersteps (q1 lags 2, q2 lags 4)
BIG = 1e30
CW = 2 + 2 * W  # row/cost tile columns: [unused, edge, interleaved 2*W]
SEG = 2 * W  # interleaved segment width (344)

_CACHE = {}


def _get_dtw_op():
    """Register (once) and return the fused DTW-row custom DVE op."""
    if "dtw_op" in _CACHE:
        return _CACHE["dtw_op"]
    import concourse.dve_ops as dve_ops_mod
    from concourse import dve_spec
    from concourse.dve_spec import Spec, Src0, Src1, C0, AluOp, scan, lower
    from concourse.dve_ops import DveOp
    from concourse.dve_uop import DveOpSpec

    def mk_scan(op, expr, init=None):
        # Scan.__post_init__ rejects a scan whose expr contains another scan,
        # but the schedule is legal (S at stage 0, b at 1-2, min-combine at 3
        # via CURR_ALU_OUT feedback); construct the node directly.
        s = object.__new__(dve_spec.Scan)
        for k, v in (("op", op), ("expr", expr), ("init", init),
                     ("_subdim_step", None)):
            object.__setattr__(s, k, v)
        return s

    def ref(in0, in1, c0, c1, c2):
        P = in0.shape[0]
        x0 = np.asarray(in0, np.float32).reshape(P, -1)
        x1 = np.asarray(in1, np.float32).reshape(P, -1)
        S = np.cumsum(x1, axis=1, dtype=np.float32)
        b = (x0 - S) + x1
        r = np.minimum.accumulate(np.minimum(b, c0), axis=1)
        return r + S

    S = scan(AluOp.ADD, Src1)
    b = (Src0 - S) + Src1
    r = mk_scan(AluOp.MIN, b, C0)
    spec = Spec(body=r + S, reference=ref)

    name = "DTW_ROW_ANT"
    if name not in dve_ops_mod._SUB_OPCODE_FOR_NAME:
        opcode = max(dve_ops_mod._SUB_OPCODE_FOR_NAME.values()) + 1
        assert opcode < 0x20
        dve_ops_mod._SUB_OPCODE_FOR_NAME[name] = opcode
    opcode = dve_ops_mod._SUB_OPCODE_FOR_NAME[name]
    shas = {}
    for ver in ("v3", "v4"):
        uops = lower(spec, ver=ver)
        shas[ver] = DveOpSpec(
            name=name, opcode=opcode, uops=uops, rd1_en=True
        ).sha(ver)
    op = DveOp(name, spec, subdim=False, uops_sha=shas)
    if all(o.name != name for o in dve_ops_mod.OPS):
        dve_ops_mod.OPS.append(op)
    dve_ops_mod.CUSTOM_DVE_SPECS[name] = spec
    _CACHE["dtw_op"] = op
    return op


def _emit(tc, v_c, w_c, out_rows):
    import concourse.bass as bass  # noqa: F401
    from concourse import mybir

    F32 = mybir.dt.float32
    nc = tc.nc
    dtw_op = _get_dtw_op()

    with (
        tc.tile_pool(name="singles", bufs=1) as singles,
        tc.tile_pool(name="wpool", bufs=6) as wpool,
        tc.tile_pool(name="psum", bufs=6, space="PSUM") as psum_pool,
    ):
        BF16 = mybir.dt.bfloat16
        # --- persistent tiles ---
        rhs = [singles.tile([KCH, L2], BF16, tag=f"rhs{g}", name=f"rhs{g}") for g in range(NCHUNK)]
        bigm = singles.tile([NQ * VB, CW], F32, tag="bigm", name="bigm")
        init0 = singles.tile([NQ * VB, 1], F32, tag="init0", name="init0")
        # row tiles: col 1 = edge/carry slot, odd cols 3,5,..,CW-1 = row values
        new = [singles.tile([NQ * VB, CW], F32, tag=f"new{p}", name=f"new{p}") for p in range(4)]
        NCC = 8  # cost-tile ring; deeper than `new` so gathers can run ahead
        cc = [singles.tile([NQ * VB, CW], F32, tag=f"c{p}", name=f"c{p}") for p in range(NCC)]

        def window(t):
            # [96, W, 2] overlapping pairs (v_{j-1}, v_j) at cols (1+2j, 3+2j)
            ap = t[:, 1:3].copy()
            ap.ap = mybir.VecI64Pair([[CW, NQ * VB], [2, W], [2, 2]])
            return ap

        # --- prologue ---
        nc.vector.memset(bigm, BIG)
        nc.vector.memset(bigm[:, 1:2], 0.0)
        nc.vector.memset(init0, 0.0)
        for p in range(4):
            nc.vector.memset(new[p][:, 1:2], BIG)
        for p in range(NCC):
            nc.vector.memset(cc[p], 0.0)
        for g in range(NCHUNK):
            nc.sync.dma_start(out=rhs[g], in_=v_c[g])

        psum_tiles = {}

        def emit_block(t):
            pt = psum_pool.tile([128, L2], F32, tag="pt", name=f"pt{t}")
            w = wpool.tile([KCH, NCHUNK * 128], BF16, tag="w", name=f"w{t}")
            nc.sync.dma_start(out=w, in_=w_c[t])
            for g in range(NCHUNK):
                nc.tensor.matmul(
                    out=pt,
                    lhsT=w[:, g * 128 : (g + 1) * 128],
                    rhs=rhs[g],
                    start=(g == 0),
                    stop=(g == NCHUNK - 1),
                )
            psum_tiles[t] = pt

        # --- wavefront: superstep s: lane q -> row s-2q cols [W*q, W*q+W) ---
        PREFETCH = 3  # emit matmul blocks this many blocks ahead of consumption
        for t in range(PREFETCH):
            emit_block(t)

        def act_gathers(s):
            # scalar-engine q0/q1 psum->cost-tile gathers for superstep s,
            # emitted one superstep early so the DP op never waits on them
            c_s = cc[s % NCC]
            if s < R:
                pt = psum_tiles[s // IBLK]
                nc.scalar.copy(
                    out=c_s[0:VB, 3:CW:2],
                    in_=pt[32 * (s % IBLK) : 32 * (s % IBLK) + 32, 0:W],
                )
            if 2 <= s < R + 2:
                ptm = psum_tiles[(s - 2) // IBLK]
                nc.scalar.copy(
                    out=c_s[VB : 2 * VB, 3:CW:2],
                    in_=ptm[32 * ((s - 2) % IBLK) : 32 * ((s - 2) % IBLK) + 32, W : 2 * W],
                )

        act_gathers(0)
        for s in range(NSS):
            if s % IBLK == 0 and s // IBLK + PREFETCH < NBLK:
                emit_block(s // IBLK + PREFETCH)
            if s + 1 < NSS:
                act_gathers(s + 1)
            c_s = cc[s % NCC]
            if 4 <= s < R + 4:
                pt2 = psum_tiles[(s - 4) // IBLK]
                nc.vector.tensor_copy(
                    out=c_s[2 * VB : 3 * VB, 3 : 3 + 2 * W2 : 2],
                    in_=pt2[32 * ((s - 4) % IBLK) : 32 * ((s - 4) % IBLK) + 32, 2 * W : L2],
                )
            nb = new[s % 4]
            if s == 0:
                pb = bigm
                ini = init0[:, 0:1]
            else:
                pb = new[(s - 1) % 4]
                if s >= 2:
                    nc.gpsimd.tensor_copy(
                        out=nb[VB : 2 * VB, 1:2],
                        in_=new[(s - 2) % 4][0:VB, CW - 1 : CW],
                    )
                if s >= 4:
                    nc.gpsimd.tensor_copy(
                        out=nb[2 * VB : 3 * VB, 1:2],
                        in_=new[(s - 2) % 4][VB : 2 * VB, CW - 1 : CW],
                    )
                if s == 2:
                    nc.vector.memset(pb[VB : 2 * VB, 2:CW], BIG)
                if s == 4:
                    nc.vector.memset(pb[2 * VB : 3 * VB, 2:CW], BIG)
                ini = nb[:, 1:2]
            nc.vector._custom_dve(
                dtw_op,
                out=nb[:, 2:CW],
                in0=window(pb),
                in1=c_s[:, 2:CW],
                s0=ini,
            )
        nc.sync.dma_start(
            out=out_rows[:, 0:SEG], in_=new[(R - 1) % 4][0:VB, 2:CW]
        )
        nc.sync.dma_start(
            out=out_rows[:, SEG : 2 * SEG],
            in_=new[(R + 1) % 4][VB : 2 * VB, 2:CW],
        )
        nc.sync.dma_start(
            out=out_rows[:, 2 * SEG : 3 * SEG],
            in_=new[(R + 3) % 4][2 * VB : 3 * VB, 2:CW],
        )


def _build():
    import concourse.bacc as bacc
    import concourse.tile as tile
    from concourse import mybir

    F32 = mybir.dt.float32
    BF16 = mybir.dt.bfloat16
    nc = bacc.Bacc()
    v_c = nc.dram_tensor("v_c", [NCHUNK, KCH, L2], BF16, kind="ExternalInput")[:]
    w_c = nc.dram_tensor("w_c", [NBLK, KCH, NCHUNK * 128], BF16, kind="ExternalInput")[:]
    out_rows = nc.dram_tensor("out_rows", [VB, 3 * SEG], F32, kind="ExternalOutput")[:]
    with tile.TileContext(nc) as tc:
        _emit(tc, v_c, w_c, out_rows)
    nc.compile()
    return nc


def _host_prep(s1, s2):
    """Build per-core v_c [5,126,512] (bf16 rhs chunks) and the full
    block-diagonal weight tensor w_c [64,126,640] (bf16, 5 K-chunks of a
    psum block side by side so one DMA loads them all)."""
    import ml_dtypes

    BF = ml_dtypes.bfloat16
    s1 = np.ascontiguousarray(s1, dtype=np.float32)
    s2 = np.ascontiguousarray(s2, dtype=np.float32)
    in_maps = []
    for c in range(N_CORES):
        s1c = s1[c * PER_CORE : (c + 1) * PER_CORE]  # [16, 512, 16]
        s2c = s2[c * PER_CORE : (c + 1) * PER_CORE]
        s1v = np.concatenate([s1c[:, :R], s1c[:, ::-1][:, :R]], axis=0)  # [32,256,16]
        s2v = np.concatenate([s2c, s2c[:, ::-1]], axis=0)  # [32,512,16]
        u = np.empty((VB, R, KAUG), np.float32)
        u[:, :, :D] = -2.0 * s1v
        u[:, :, D] = 1.0
        u[:, :, D + 1] = (s1v * s1v).sum(-1)
        v = np.empty((VB, L2, KAUG), np.float32)
        v[:, :, :D] = s2v
        v[:, :, D] = (s2v * s2v).sum(-1)
        v[:, :, D + 1] = 1.0
        u = u.astype(BF)
        vch = np.zeros((NCHUNK, KCH, L2), BF)
        wch = np.zeros((NBLK, NCHUNK, KCH, 128), BF)
        for g in range(NCHUNK):
            for vl in range(min(7, VB - 7 * g)):
                vb = 7 * g + vl
                vch[g, vl * KAUG : (vl + 1) * KAUG, :] = v[vb].T
                # w[t, g, vl*18+d, il*32+vb] = u[vb, 4t+il, d]
                wch[:, g, vl * KAUG : (vl + 1) * KAUG, vb::VB] = (
                    u[vb].reshape(NBLK, IBLK, KAUG).transpose(0, 2, 1)
                )
        w2 = np.ascontiguousarray(wch.transpose(0, 2, 1, 3)).reshape(
            NBLK, KCH, NCHUNK * 128
        )
        in_maps.append(
            {
                "v_c": vch,
                "w_c": w2,
            }
        )
    return in_maps


def _combine(outs):
    """outs: list of [VB, 3*SEG] interleaved final-row arrays per core ->
    scalar loss. Row value j of segment q sits at col q*SEG + 1 + 2j."""
    vals = np.empty(B, np.float64)
    for c in range(N_CORES):
        rows = np.asarray(outs[c], np.float64)
        full = np.empty((VB, L2), np.float64)
        full[:, 0:W] = rows[:, 1 : 2 * W : 2]
        full[:, W : 2 * W] = rows[:, SEG + 1 : SEG + 2 * W : 2]
        full[:, 2 * W : L2] = rows[:, 2 * SEG + 1 : 2 * SEG + 2 * W2 : 2]
        for bl in range(PER_CORE):
            F = full[bl]
            Brow = full[PER_CORE + bl][::-1]
            Bnext = np.concatenate([Brow[1:], [np.inf]])
            vals[c * PER_CORE + bl] = np.min(F + np.minimum(Brow, Bnext))
    return np.float32(np.mean(np.sqrt(vals)))


def kernel(s1_batch, s2_batch):
    from concourse import bass_utils

    if "nc" not in _CACHE:
        _CACHE["nc"] = _build()
    nc = _CACHE["nc"]
    in_maps = _host_prep(np.asarray(s1_batch), np.asarray(s2_batch))
    kw = {}
    if _CACHE.get("trace"):
        kw = dict(trace=True, trace_cores=_CACHE.get("trace_cores", [0]),
                  tmpdir=_CACHE.get("tmpdir"))
    res = bass_utils.run_bass_kernel_spmd(
        nc, in_maps, core_ids=list(range(N_CORES)), **kw
    )
    if res.exec_time_ns is not None:
        _CACHE["exec_time_ns"] = res.exec_time_ns
    _CACHE["last_results"] = res
    outs = [r["out_rows"] for r in res.results]
    return _combine(outs)


# revision 14
# speedup vs baseline: 1.2766x; 1.0085x over previous
"""DTW loss kernel for Trainium2 (8 NeuronCores, Bass/Tile).

Strategy
--------
reference: C[b,i,j] = ||s1[b,i]-s2[b,j]||^2 ; DTW DP over [512,512]; return
mean_b sqrt(DTW[b,-1,-1]).

Meet-in-the-middle: any monotone DTW path crosses the row-255/256 boundary
exactly once, so DTW_end = min_j F[255,j] + min(B[256,j], B[256,j+1]) where F
is the forward DP over rows 0..255 and B the backward DP (a forward DP on the
reversed sequences). Each core handles 16 batch elements * 2 directions = 32
independent half-DPs ("virtual batches", vb) of 256 rows. A 3-block column
wavefront runs on 96 partitions = (q, vb): at superstep s lane (q,vb) scans
row s-2q over cols [W*q, W*q+W).

Each DP row is ONE custom DVE instruction (full-rate 1 elem/cycle, vs the
stock tensor_tensor_scan's half-rate bubble path) via a change of variables:
with S_j = prefix-sum(c), the recurrence state_j = min(prev_j, prev_{j-1},
state_{j-1}) + c_j becomes a pure running min state'_j = min(state'_{j-1},
m_j - S_{j-1}) with state_j = state'_j + S_j. Feeding the op an overlapping
window AP (prev_{j-1}, prev_j interleaved, stride-2 pairs) and a
zero-interleaved cost stream makes m_j's two terms arrive as separate stream
elements, so the body is uniform:
    S = scan(ADD, Src1); b = Src0 - S + Src1
    r = scan(MIN, b, init=C0); out = r + S
Row tiles store values at odd columns (col 1 = left-edge/carry slot), so one
op's output is directly the next op's window input. The per-lane carry init
C0 rides in a [96,1] column AP written by one gpsimd copy per superstep.

The cost rows are made on the PE in bf16: C[vb,i,j] = u[vb,i,:]@v[vb,j,:]
with u = [-2*s1, 1, |s1|^2], v = [s2, |s2|^2, 1] (K=18), batched over vb via
block-diagonal weights. All 5 K-chunk weight tiles for a psum block load with
ONE dma (dram layout [KCH, 5*128]) to keep the sync sequencer off the
critical path. The scalar engine gathers q0/q1 psum pieces into the
wavefront cost tiles (strided odd-column writes); the vector engine gathers
q2's.
"""

import numpy as np

B = 128
L1 = 512
L2 = 512
D = 16
N_CORES = 8
PER_CORE = B // N_CORES  # 16
VB = 2 * PER_CORE  # 32 virtual batches (fwd+bwd)
R = L1 // 2  # 256 rows per half-DP
KAUG = D + 2  # 18
NCHUNK = 5  # matmul chunks of up to 7 vb, K rows = 7*18 = 126 (unpadded)
KCH = 126  # K rows per chunk
IBLK = 4  # DP rows per psum block
NBLK = R // IBLK  # 64
NQ = 3  # wavefront j-blocks
W = 172  # block width (3*172 = 516; q2 has 4 virtual pad cols)
W2 = L2 - 2 * W  # 168 real cols in q2's block
NSS = R + 4  # 260 supersteps (q1 lags 2, q2 lags 4)
BIG = 1e30
CW = 2 + 2 * W  # row/cost tile columns: [unused, edge, interleaved 2*W]
SEG = 2 * W  # interleaved segment width (344)

_CACHE = {}


def _get_dtw_op():
    """Register (once) and return the fused DTW-row custom DVE op."""
    if "dtw_op" in _CACHE:
        return _CACHE["dtw_op"]
    import concourse.dve_ops as dve_ops_mod
    from concourse import dve_spec
    from concourse.dve_spec import Spec, Src0, Src1, C0, AluOp, scan, lower
    from concourse.dve_ops import DveOp
    from concourse.dve_uop import DveOpSpec

    def mk_scan(op, expr, init=None):
        # Scan.__post_init__ rejects a scan whose expr contains another scan,
        # but the schedule is legal (S at stage 0, b at 1-2, min-combine at 3
        # via CURR_ALU_OUT feedback); construct the node directly.
        s = object.__new__(dve_spec.Scan)
        for k, v in (("op", op), ("expr", expr), ("init", init),
                     ("_subdim_step", None)):
            object.__setattr__(s, k, v)
        return s

    def ref(in0, in1, c0, c1, c2):
        P = in0.shape[0]
        x0 = np.asarray(in0, np.float32).reshape(P, -1)
        x1 = np.asarray(in1, np.float32).reshape(P, -1)
        S = np.cumsum(x1, axis=1, dtype=np.float32)
        b = (x0 - S) + x1
        r = np.minimum.accumulate(np.minimum(b, c0), axis=1)
        return r + S

    S = scan(AluOp.ADD, Src1)
    b = (Src0 - S) + Src1
    r = mk_scan(AluOp.MIN, b, C0)
    spec = Spec(body=r + S, reference=ref)

    name = "DTW_ROW_ANT"
    if name not in dve_ops_mod._SUB_OPCODE_FOR_NAME:
        opcode = max(dve_ops_mod._SUB_OPCODE_FOR_NAME.values()) + 1
        assert opcode < 0x20
        dve_ops_mod._SUB_OPCODE_FOR_NAME[name] = opcode
    opcode = dve_ops_mod._SUB_OPCODE_FOR_NAME[name]
    shas = {}
    for ver in ("v3", "v4"):
        uops = lower(spec, ver=ver)
        shas[ver] = DveOpSpec(
            name=name, opcode=opcode, uops=uops, rd1_en=True
        ).sha(ver)
    op = DveOp(name, spec, subdim=False, uops_sha=shas)
    if all(o.name != name for o in dve_ops_mod.OPS):
        dve_ops_mod.OPS.append(op)
    dve_ops_mod.CUSTOM_DVE_SPECS[name] = spec
    _CACHE["dtw_op"] = op
    return op


def _emit(tc, v_c, w_c, out_rows):
    import concourse.bass as bass  # noqa: F401
    from concourse import mybir

    F32 = mybir.dt.float32
    nc = tc.nc
    dtw_op = _get_dtw_op()

    with (
        tc.tile_pool(name="singles", bufs=1) as singles,
        tc.tile_pool(name="wpool", bufs=6) as wpool,
        tc.tile_pool(name="psum", bufs=6, space="PSUM") as psum_pool,
    ):
        BF16 = mybir.dt.bfloat16
        # --- persistent tiles ---
        rhs = [singles.tile([KCH, L2], BF16, tag=f"rhs{g}", name=f"rhs{g}") for g in range(NCHUNK)]
        bigm = singles.tile([NQ * VB, CW], F32, tag="bigm", name="bigm")
        init0 = singles.tile([NQ * VB, 1], F32, tag="init0", name="init0")
        # row tiles: col 1 = edge/carry slot, odd cols 3,5,..,CW-1 = row values
        new = [singles.tile([NQ * VB, CW], F32, tag=f"new{p}", name=f"new{p}") for p in range(4)]
        NCC = 8  # cost-tile ring; deeper than `new` so gathers can run ahead
        cc = [singles.tile([NQ * VB, CW], F32, tag=f"c{p}", name=f"c{p}") for p in range(NCC)]

        def window(t):
            # [96, W, 2] overlapping pairs (v_{j-1}, v_j) at cols (1+2j, 3+2j)
            ap = t[:, 1:3].copy()
            ap.ap = mybir.VecI64Pair([[CW, NQ * VB], [2, W], [2, 2]])
            return ap

        # --- prologue ---
        nc.vector.memset(bigm, BIG)
        nc.vector.memset(bigm[:, 1:2], 0.0)
        nc.vector.memset(init0, 0.0)
        for p in range(4):
            nc.vector.memset(new[p][:, 1:2], BIG)
        for p in range(NCC):
            nc.vector.memset(cc[p], 0.0)
        # rhs[0] gates the first matmul: split it over 4 queues so its
        # transfer isn't a single-queue 6us serial wait at the head
        for k in range(4):
            nc.sync.dma_start(
                out=rhs[0][:, 128 * k : 128 * (k + 1)],
                in_=v_c[0, :, 128 * k : 128 * (k + 1)],
            )
        for g in range(1, NCHUNK):
            nc.sync.dma_start(out=rhs[g], in_=v_c[g])

        psum_tiles = {}

        def emit_block(t):
            pt = psum_pool.tile([128, L2], F32, tag="pt", name=f"pt{t}")
            w = wpool.tile([KCH, NCHUNK * 128], BF16, tag="w", name=f"w{t}")
            if t == 0:
                # head-latency: chunked load lets matmul g start after chunk g
                for g in range(NCHUNK):
                    nc.sync.dma_start(
                        out=w[:, 128 * g : 128 * (g + 1)],
                        in_=w_c[t, :, 128 * g : 128 * (g + 1)],
                    )
            else:
                nc.sync.dma_start(out=w, in_=w_c[t])
            for g in range(NCHUNK):
                nc.tensor.matmul(
                    out=pt,
                    lhsT=w[:, g * 128 : (g + 1) * 128],
                    rhs=rhs[g],
                    start=(g == 0),
                    stop=(g == NCHUNK - 1),
                )
            psum_tiles[t] = pt

        # --- wavefront: superstep s: lane q -> row s-2q cols [W*q, W*q+W) ---
        PREFETCH = 3  # emit matmul blocks this many blocks ahead of consumption
        for t in range(PREFETCH):
            emit_block(t)

        def act_gathers(s):
            # scalar-engine q0/q1 psum->cost-tile gathers for superstep s,
            # emitted one superstep early so the DP op never waits on them
            c_s = cc[s % NCC]
            if s < R:
                pt = psum_tiles[s // IBLK]
                nc.scalar.copy(
                    out=c_s[0:VB, 3:CW:2],
                    in_=pt[32 * (s % IBLK) : 32 * (s % IBLK) + 32, 0:W],
                )
            if 2 <= s < R + 2:
                ptm = psum_tiles[(s - 2) // IBLK]
                nc.scalar.copy(
                    out=c_s[VB : 2 * VB, 3:CW:2],
                    in_=ptm[32 * ((s - 2) % IBLK) : 32 * ((s - 2) % IBLK) + 32, W : 2 * W],
                )

        act_gathers(0)
        for s in range(NSS):
            if s % IBLK == 0 and s // IBLK + PREFETCH < NBLK:
                emit_block(s // IBLK + PREFETCH)
            if s + 1 < NSS:
                act_gathers(s + 1)
            c_s = cc[s % NCC]
            if 4 <= s < R + 4:
                pt2 = psum_tiles[(s - 4) // IBLK]
                # q2 gather rides the vector engine, except every 8th
                # superstep on the scalar engine to balance their loads
                eng_copy = nc.scalar.copy if s % 8 == 7 else nc.vector.tensor_copy
                eng_copy(
                    out=c_s[2 * VB : 3 * VB, 3 : 3 + 2 * W2 : 2],
                    in_=pt2[32 * ((s - 4) % IBLK) : 32 * ((s - 4) % IBLK) + 32, 2 * W : L2],
                )
            nb = new[s % 4]
            if s == 0:
                pb = bigm
                ini = init0[:, 0:1]
            else:
                pb = new[(s - 1) % 4]
                if s >= 2:
                    nc.gpsimd.tensor_copy(
                        out=nb[VB : 2 * VB, 1:2],
                        in_=new[(s - 2) % 4][0:VB, CW - 1 : CW],
                    )
                if s >= 4:
                    nc.gpsimd.tensor_copy(
                        out=nb[2 * VB : 3 * VB, 1:2],
                        in_=new[(s - 2) % 4][VB : 2 * VB, CW - 1 : CW],
                    )
                if s == 2:
                    nc.vector.memset(pb[VB : 2 * VB, 2:CW], BIG)
                if s == 4:
                    nc.vector.memset(pb[2 * VB : 3 * VB, 2:CW], BIG)
                ini = nb[:, 1:2]
            nc.vector._custom_dve(
                dtw_op,
                out=nb[:, 2:CW],
                in0=window(pb),
                in1=c_s[:, 2:CW],
                s0=ini,
            )
        nc.sync.dma_start(
            out=out_rows[:, 0:SEG], in_=new[(R - 1) % 4][0:VB, 2:CW]
        )
        nc.sync.dma_start(
            out=out_rows[:, SEG : 2 * SEG],
            in_=new[(R + 1) % 4][VB : 2 * VB, 2:CW],
        )
        nc.sync.dma_start(
            out=out_rows[:, 2 * SEG : 3 * SEG],
            in_=new[(R + 3) % 4][2 * VB : 3 * VB, 2:CW],
        )


def _build():
    import concourse.bacc as bacc
    import concourse.tile as tile
    from concourse import mybir

    F32 = mybir.dt.float32
    BF16 = mybir.dt.bfloat16
    nc = bacc.Bacc()
    v_c = nc.dram_tensor("v_c", [NCHUNK, KCH, L2], BF16, kind="ExternalInput")[:]
    w_c = nc.dram_tensor("w_c", [NBLK, KCH, NCHUNK * 128], BF16, kind="ExternalInput")[:]
    out_rows = nc.dram_tensor("out_rows", [VB, 3 * SEG], F32, kind="ExternalOutput")[:]
    with tile.TileContext(nc) as tc:
        _emit(tc, v_c, w_c, out_rows)
    nc.compile()
    return nc


def _host_prep(s1, s2):
    """Build per-core v_c [5,126,512] (bf16 rhs chunks) and the full
    block-diagonal weight tensor w_c [64,126,640] (bf16, 5 K-chunks of a
    psum block side by side so one DMA loads them all)."""
    import ml_dtypes

    BF = ml_dtypes.bfloat16
    s1 = np.ascontiguousarray(s1, dtype=np.float32)
    s2 = np.ascontiguousarray(s2, dtype=np.float32)
    in_maps = []
    for c in range(N_CORES):
        s1c = s1[c * PER_CORE : (c + 1) * PER_CORE]  # [16, 512, 16]
        s2c = s2[c * PER_CORE : (c + 1) * PER_CORE]
        s1v = np.concatenate([s1c[:, :R], s1c[:, ::-1][:, :R]], axis=0)  # [32,256,16]
        s2v = np.concatenate([s2c, s2c[:, ::-1]], axis=0)  # [32,512,16]
        u = np.empty((VB, R, KAUG), np.float32)
        u[:, :, :D] = -2.0 * s1v
        u[:, :, D] = 1.0
        u[:, :, D + 1] = (s1v * s1v).sum(-1)
        v = np.empty((VB, L2, KAUG), np.float32)
        v[:, :, :D] = s2v
        v[:, :, D] = (s2v * s2v).sum(-1)
        v[:, :, D + 1] = 1.0
        u = u.astype(BF)
        vch = np.zeros((NCHUNK, KCH, L2), BF)
        wch = np.zeros((NBLK, NCHUNK, KCH, 128), BF)
        for g in range(NCHUNK):
            for vl in range(min(7, VB - 7 * g)):
                vb = 7 * g + vl
                vch[g, vl * KAUG : (vl + 1) * KAUG, :] = v[vb].T
                # w[t, g, vl*18+d, il*32+vb] = u[vb, 4t+il, d]
                wch[:, g, vl * KAUG : (vl + 1) * KAUG, vb::VB] = (
                    u[vb].reshape(NBLK, IBLK, KAUG).transpose(0, 2, 1)
                )
        w2 = np.ascontiguousarray(wch.transpose(0, 2, 1, 3)).reshape(
            NBLK, KCH, NCHUNK * 128
        )
        in_maps.append(
            {
                "v_c": vch,
                "w_c": w2,
            }
        )
    return in_maps


def _combine(outs):
    """outs: list of [VB, 3*SEG] interleaved final-row arrays per core ->
    scalar loss. Row value j of segment q sits at col q*SEG + 1 + 2j."""
    vals = np.empty(B, np.float64)
    for c in range(N_CORES):
        rows = np.asarray(outs[c], np.float64)
        full = np.empty((VB, L2), np.float64)
        full[:, 0:W] = rows[:, 1 : 2 * W : 2]
        full[:, W : 2 * W] = rows[:, SEG + 1 : SEG + 2 * W : 2]
        full[:, 2 * W : L2] = rows[:, 2 * SEG + 1 : 2 * SEG + 2 * W2 : 2]
        for bl in range(PER_CORE):
            F = full[bl]
            Brow = full[PER_CORE + bl][::-1]
            Bnext = np.concatenate([Brow[1:], [np.inf]])
            vals[c * PER_CORE + bl] = np.min(F + np.minimum(Brow, Bnext))
    return np.float32(np.mean(np.sqrt(vals)))


def kernel(s1_batch, s2_batch):
    from concourse import bass_utils

    if "nc" not in _CACHE:
        _CACHE["nc"] = _build()
    nc = _CACHE["nc"]
    in_maps = _host_prep(np.asarray(s1_batch), np.asarray(s2_batch))
    kw = {}
    if _CACHE.get("trace"):
        kw = dict(trace=True, trace_cores=_CACHE.get("trace_cores", [0]),
                  tmpdir=_CACHE.get("tmpdir"))
    res = bass_utils.run_bass_kernel_spmd(
        nc, in_maps, core_ids=list(range(N_CORES)), **kw
    )
    if res.exec_time_ns is not None:
        _CACHE["exec_time_ns"] = res.exec_time_ns
    _CACHE["last_results"] = res
    outs = [r["out_rows"] for r in res.results]
    return _combine(outs)


# revision 21
# speedup vs baseline: 1.2812x; 1.0036x over previous
"""DTW loss kernel for Trainium2 (8 NeuronCores, Bass/Tile).

Strategy
--------
reference: C[b,i,j] = ||s1[b,i]-s2[b,j]||^2 ; DTW DP over [512,512]; return
mean_b sqrt(DTW[b,-1,-1]).

Meet-in-the-middle: any monotone DTW path crosses the row-255/256 boundary
exactly once, so DTW_end = min_j F[255,j] + min(B[256,j], B[256,j+1]) where F
is the forward DP over rows 0..255 and B the backward DP (a forward DP on the
reversed sequences). Each core handles 16 batch elements * 2 directions = 32
independent half-DPs ("virtual batches", vb) of 256 rows. A 3-block column
wavefront runs on 96 partitions = (q, vb): at superstep s lane (q,vb) scans
row s-2q over cols [W*q, W*q+W).

Each DP row is ONE custom DVE instruction (full-rate 1 elem/cycle, vs the
stock tensor_tensor_scan's half-rate bubble path) via a change of variables:
with S_j = prefix-sum(c), the recurrence state_j = min(prev_j, prev_{j-1},
state_{j-1}) + c_j becomes a pure running min state'_j = min(state'_{j-1},
m_j - S_{j-1}) with state_j = state'_j + S_j. Feeding the op an overlapping
window AP (prev_{j-1}, prev_j interleaved, stride-2 pairs) and a
zero-interleaved cost stream makes m_j's two terms arrive as separate stream
elements, so the body is uniform:
    S = scan(ADD, Src1); b = Src0 - S + Src1
    r = scan(MIN, b, init=C0); out = r + S
Row tiles store values at odd columns (col 1 = left-edge/carry slot), so one
op's output is directly the next op's window input. The per-lane carry init
C0 rides in a [96,1] column AP written by one gpsimd copy per superstep.

The cost rows are made on the PE in bf16: C[vb,i,j] = u[vb,i,:]@v[vb,j,:]
with u = [-2*s1, 1, |s1|^2], v = [s2, |s2|^2, 1] (K=18), batched over vb via
block-diagonal weights. All 5 K-chunk weight tiles for a psum block load with
ONE dma (dram layout [KCH, 5*128]) to keep the sync sequencer off the
critical path. The scalar engine gathers q0/q1 psum pieces into the
wavefront cost tiles (strided odd-column writes); the vector engine gathers
q2's.
"""

import numpy as np

B = 128
L1 = 512
L2 = 512
D = 16
N_CORES = 8
PER_CORE = B // N_CORES  # 16
VB = 2 * PER_CORE  # 32 virtual batches (fwd+bwd)
R = L1 // 2  # 256 rows per half-DP
KAUG = D + 2  # 18
NCHUNK = 5  # matmul chunks of up to 7 vb, K rows = 7*18 = 126 (unpadded)
KCH = 126  # K rows per chunk
IBLK = 4  # DP rows per psum block
NBLK = R // IBLK  # 64
NQ = 3  # wavefront j-blocks
W = 172  # block width (3*172 = 516; q2 has 4 virtual pad cols)
W2 = L2 - 2 * W  # 168 real cols in q2's block
NSS = R + 4  # 260 supersteps (q1 lags 2, q2 lags 4)
BIG = 1e30
CW = 2 + 2 * W  # row/cost tile columns: [unused, edge, interleaved 2*W]
SEG = 2 * W  # interleaved segment width (344)

_CACHE = {}


def _get_dtw_op():
    """Register (once) and return the fused DTW-row custom DVE op."""
    if "dtw_op" in _CACHE:
        return _CACHE["dtw_op"]
    import concourse.dve_ops as dve_ops_mod
    from concourse import dve_spec
    from concourse.dve_spec import Spec, Src0, Src1, C0, AluOp, scan, lower
    from concourse.dve_ops import DveOp
    from concourse.dve_uop import DveOpSpec

    def mk_scan(op, expr, init=None):
        # Scan.__post_init__ rejects a scan whose expr contains another scan,
        # but the schedule is legal (S at stage 0, b at 1-2, min-combine at 3
        # via CURR_ALU_OUT feedback); construct the node directly.
        s = object.__new__(dve_spec.Scan)
        for k, v in (("op", op), ("expr", expr), ("init", init),
                     ("_subdim_step", None)):
            object.__setattr__(s, k, v)
        return s

    def ref(in0, in1, c0, c1, c2):
        P = in0.shape[0]
        x0 = np.asarray(in0, np.float32).reshape(P, -1)
        x1 = np.asarray(in1, np.float32).reshape(P, -1)
        S = np.cumsum(x1, axis=1, dtype=np.float32)
        b = (x0 - S) + x1
        r = np.minimum.accumulate(np.minimum(b, c0), axis=1)
        return r + S

    S = scan(AluOp.ADD, Src1)
    b = (Src0 - S) + Src1
    r = mk_scan(AluOp.MIN, b, C0)
    spec = Spec(body=r + S, reference=ref)

    name = "DTW_ROW_ANT"
    if name not in dve_ops_mod._SUB_OPCODE_FOR_NAME:
        opcode = max(dve_ops_mod._SUB_OPCODE_FOR_NAME.values()) + 1
        assert opcode < 0x20
        dve_ops_mod._SUB_OPCODE_FOR_NAME[name] = opcode
    opcode = dve_ops_mod._SUB_OPCODE_FOR_NAME[name]
    shas = {}
    for ver in ("v3", "v4"):
        uops = lower(spec, ver=ver)
        shas[ver] = DveOpSpec(
            name=name, opcode=opcode, uops=uops, rd1_en=True
        ).sha(ver)
    op = DveOp(name, spec, subdim=False, uops_sha=shas)
    if all(o.name != name for o in dve_ops_mod.OPS):
        dve_ops_mod.OPS.append(op)
    dve_ops_mod.CUSTOM_DVE_SPECS[name] = spec
    _CACHE["dtw_op"] = op
    return op


def _emit(tc, v_c, w_c, out_rows):
    import concourse.bass as bass  # noqa: F401
    from concourse import mybir

    F32 = mybir.dt.float32
    nc = tc.nc
    dtw_op = _get_dtw_op()

    with (
        tc.tile_pool(name="singles", bufs=1) as singles,
        tc.tile_pool(name="wpool", bufs=6) as wpool,
        tc.tile_pool(name="psum", bufs=6, space="PSUM") as psum_pool,
    ):
        BF16 = mybir.dt.bfloat16
        # --- persistent tiles ---
        rhs = [singles.tile([KCH, L2], BF16, tag=f"rhs{g}", name=f"rhs{g}") for g in range(NCHUNK)]
        bigm = singles.tile([NQ * VB, CW], F32, tag="bigm", name="bigm")
        q0save = singles.tile([VB, CW], F32, tag="q0save", name="q0save")
        init0 = singles.tile([NQ * VB, 1], F32, tag="init0", name="init0")
        # row tiles: col 1 = edge/carry slot, odd cols 3,5,..,CW-1 = row values
        new = [singles.tile([NQ * VB, CW], F32, tag=f"new{p}", name=f"new{p}") for p in range(4)]
        NCC = 8  # cost-tile ring; deeper than `new` so gathers can run ahead
        cc = [singles.tile([NQ * VB, CW], F32, tag=f"c{p}", name=f"c{p}") for p in range(NCC)]

        def window(t, lane_lo=0):
            # [P, W, 2] overlapping pairs (v_{j-1}, v_j) at cols (1+2j, 3+2j)
            ap = t[lane_lo : NQ * VB, 1:3].copy()
            ap.ap = mybir.VecI64Pair([[CW, NQ * VB - lane_lo], [2, W], [2, 2]])
            return ap

        # --- prologue ---
        nc.vector.memset(bigm, BIG)
        nc.vector.memset(bigm[:, 1:2], 0.0)
        nc.vector.memset(init0, 0.0)
        for p in range(4):
            nc.vector.memset(new[p][:, 1:2], BIG)
        for p in range(NCC):
            nc.vector.memset(cc[p], 0.0)
        # rhs[0] gates the first matmul: split it over 4 queues so its
        # transfer isn't a single-queue 6us serial wait at the head
        for k in range(4):
            nc.sync.dma_start(
                out=rhs[0][:, 128 * k : 128 * (k + 1)],
                in_=v_c[0, :, 128 * k : 128 * (k + 1)],
            )
        for g in range(1, NCHUNK):
            nc.sync.dma_start(out=rhs[g], in_=v_c[g])

        psum_tiles = {}

        def emit_block(t):
            pt = psum_pool.tile([128, L2], F32, tag="pt", name=f"pt{t}")
            w = wpool.tile([KCH, NCHUNK * 128], BF16, tag="w", name=f"w{t}")
            if t == 0:
                # head-latency: chunked load lets matmul g start after chunk g
                for g in range(NCHUNK):
                    nc.sync.dma_start(
                        out=w[:, 128 * g : 128 * (g + 1)],
                        in_=w_c[t, :, 128 * g : 128 * (g + 1)],
                    )
            else:
                nc.sync.dma_start(out=w, in_=w_c[t])
            for g in range(NCHUNK):
                nc.tensor.matmul(
                    out=pt,
                    lhsT=w[:, g * 128 : (g + 1) * 128],
                    rhs=rhs[g],
                    start=(g == 0),
                    stop=(g == NCHUNK - 1),
                )
            psum_tiles[t] = pt

        # --- wavefront: superstep s: lane q -> row s-2q cols [W*q, W*q+W) ---
        PREFETCH = 3  # emit matmul blocks this many blocks ahead of consumption
        for t in range(PREFETCH):
            emit_block(t)

        def act_gathers(s):
            # scalar-engine q0/q1 psum->cost-tile gathers for superstep s,
            # emitted one superstep early so the DP op never waits on them
            c_s = cc[s % NCC]
            if s < R:
                pt = psum_tiles[s // IBLK]
                nc.scalar.copy(
                    out=c_s[0:VB, 3:CW:2],
                    in_=pt[32 * (s % IBLK) : 32 * (s % IBLK) + 32, 0:W],
                )
            if 2 <= s < R + 2:
                ptm = psum_tiles[(s - 2) // IBLK]
                nc.scalar.copy(
                    out=c_s[VB : 2 * VB, 3:CW:2],
                    in_=ptm[32 * ((s - 2) % IBLK) : 32 * ((s - 2) % IBLK) + 32, W : 2 * W],
                )

        act_gathers(0)
        act_gathers(1)
        for s in range(NSS):
            if s % IBLK == 0 and s // IBLK + PREFETCH < NBLK:
                emit_block(s // IBLK + PREFETCH)
            if s + 2 < NSS:
                act_gathers(s + 2)
            c_s = cc[s % NCC]
            if 4 <= s < R + 4:
                pt2 = psum_tiles[(s - 4) // IBLK]
                # q2 gather rides the vector engine, except every 8th
                # superstep on the scalar engine to balance their loads
                eng_copy = nc.scalar.copy if s % 8 == 7 else nc.vector.tensor_copy
                eng_copy(
                    out=c_s[2 * VB : 3 * VB, 3 : 3 + 2 * W2 : 2],
                    in_=pt2[32 * ((s - 4) % IBLK) : 32 * ((s - 4) % IBLK) + 32, 2 * W : L2],
                )
            nb = new[s % 4]
            if s == 0:
                pb = bigm
                ini = init0[:, 0:1]
            else:
                pb = new[(s - 1) % 4]
                if 2 <= s < R + 2:
                    nc.gpsimd.tensor_copy(
                        out=nb[VB : 2 * VB, 1:2],
                        in_=new[(s - 2) % 4][0:VB, CW - 1 : CW],
                    )
                if 4 <= s < R + 4:
                    nc.gpsimd.tensor_copy(
                        out=nb[2 * VB : 3 * VB, 1:2],
                        in_=new[(s - 2) % 4][VB : 2 * VB, CW - 1 : CW],
                    )
                if s == 2:
                    nc.vector.memset(pb[VB : 2 * VB, 2:CW], BIG)
                if s == 4:
                    nc.vector.memset(pb[2 * VB : 3 * VB, 2:CW], BIG)
                ini = nb[:, 1:2]
            nc.vector._custom_dve(
                dtw_op,
                out=nb[:, 2:CW],
                in0=window(pb),
                in1=c_s[:, 2:CW],
                s0=ini,
            )
            if s == R:
                # rescue q0's final row before the s=R+3 op (same tile ring
                # slot, all 96 lanes) overwrites it; emission order makes the
                # overwriting op wait for this read
                nc.scalar.copy(
                    out=q0save[:, 2:CW], in_=new[(R - 1) % 4][0:VB, 2:CW]
                )
        nc.sync.dma_start(out=out_rows[:, 0:SEG], in_=q0save[:, 2:CW])
        nc.sync.dma_start(
            out=out_rows[:, SEG : 2 * SEG],
            in_=new[(R + 1) % 4][VB : 2 * VB, 2:CW],
        )
        nc.sync.dma_start(
            out=out_rows[:, 2 * SEG : 3 * SEG],
            in_=new[(R + 3) % 4][2 * VB : 3 * VB, 2:CW],
        )


def _build():
    import concourse.bacc as bacc
    import concourse.tile as tile
    from concourse import mybir

    F32 = mybir.dt.float32
    BF16 = mybir.dt.bfloat16
    nc = bacc.Bacc()
    v_c = nc.dram_tensor("v_c", [NCHUNK, KCH, L2], BF16, kind="ExternalInput")[:]
    w_c = nc.dram_tensor("w_c", [NBLK, KCH, NCHUNK * 128], BF16, kind="ExternalInput")[:]
    out_rows = nc.dram_tensor("out_rows", [VB, 3 * SEG], F32, kind="ExternalOutput")[:]
    with tile.TileContext(nc) as tc:
        _emit(tc, v_c, w_c, out_rows)
    nc.compile()
    return nc


def _host_prep(s1, s2):
    """Build per-core v_c [5,126,512] (bf16 rhs chunks) and the full
    block-diagonal weight tensor w_c [64,126,640] (bf16, 5 K-chunks of a
    psum block side by side so one DMA loads them all)."""
    import ml_dtypes

    BF = ml_dtypes.bfloat16
    s1 = np.ascontiguousarray(s1, dtype=np.float32)
    s2 = np.ascontiguousarray(s2, dtype=np.float32)
    in_maps = []
    for c in range(N_CORES):
        s1c = s1[c * PER_CORE : (c + 1) * PER_CORE]  # [16, 512, 16]
        s2c = s2[c * PER_CORE : (c + 1) * PER_CORE]
        s1v = np.concatenate([s1c[:, :R], s1c[:, ::-1][:, :R]], axis=0)  # [32,256,16]
        s2v = np.concatenate([s2c, s2c[:, ::-1]], axis=0)  # [32,512,16]
        u = np.empty((VB, R, KAUG), np.float32)
        u[:, :, :D] = -2.0 * s1v
        u[:, :, D] = 1.0
        u[:, :, D + 1] = (s1v * s1v).sum(-1)
        v = np.empty((VB, L2, KAUG), np.float32)
        v[:, :, :D] = s2v
        v[:, :, D] = (s2v * s2v).sum(-1)
        v[:, :, D + 1] = 1.0
        u = u.astype(BF)
        vch = np.zeros((NCHUNK, KCH, L2), BF)
        wch = np.zeros((NBLK, NCHUNK, KCH, 128), BF)
        for g in range(NCHUNK):
            for vl in range(min(7, VB - 7 * g)):
                vb = 7 * g + vl
                vch[g, vl * KAUG : (vl + 1) * KAUG, :] = v[vb].T
                # w[t, g, vl*18+d, il*32+vb] = u[vb, 4t+il, d]
                wch[:, g, vl * KAUG : (vl + 1) * KAUG, vb::VB] = (
                    u[vb].reshape(NBLK, IBLK, KAUG).transpose(0, 2, 1)
                )
        w2 = np.ascontiguousarray(wch.transpose(0, 2, 1, 3)).reshape(
            NBLK, KCH, NCHUNK * 128
        )
        in_maps.append(
            {
                "v_c": vch,
                "w_c": w2,
            }
        )
    return in_maps


def _combine(outs):
    """outs: list of [VB, 3*SEG] interleaved final-row arrays per core ->
    scalar loss. Row value j of segment q sits at col q*SEG + 1 + 2j."""
    vals = np.empty(B, np.float64)
    for c in range(N_CORES):
        rows = np.asarray(outs[c], np.float64)
        full = np.empty((VB, L2), np.float64)
        full[:, 0:W] = rows[:, 1 : 2 * W : 2]
        full[:, W : 2 * W] = rows[:, SEG + 1 : SEG + 2 * W : 2]
        full[:, 2 * W : L2] = rows[:, 2 * SEG + 1 : 2 * SEG + 2 * W2 : 2]
        for bl in range(PER_CORE):
            F = full[bl]
            Brow = full[PER_CORE + bl][::-1]
            Bnext = np.concatenate([Brow[1:], [np.inf]])
            vals[c * PER_CORE + bl] = np.min(F + np.minimum(Brow, Bnext))
    return np.float32(np.mean(np.sqrt(vals)))


def kernel(s1_batch, s2_batch):
    from concourse import bass_utils

    if "nc" not in _CACHE:
        _CACHE["nc"] = _build()
    nc = _CACHE["nc"]
    in_maps = _host_prep(np.asarray(s1_batch), np.asarray(s2_batch))
    kw = {}
    if _CACHE.get("trace"):
        kw = dict(trace=True, trace_cores=_CACHE.get("trace_cores", [0]),
                  tmpdir=_CACHE.get("tmpdir"))
    res = bass_utils.run_bass_kernel_spmd(
        nc, in_maps, core_ids=list(range(N_CORES)), **kw
    )
    if res.exec_time_ns is not None:
        _CACHE["exec_time_ns"] = res.exec_time_ns
    _CACHE["last_results"] = res
    outs = [r["out_rows"] for r in res.results]
    return _combine(outs)
